# revision 68
# baseline (speedup 1.0000x reference)
"""Trainium2 Bass kernel for nn_Block_85598698209846 (moe_routing).

Strategy (8 NeuronCores, SPMD single program, per-core data):
- Tokens are assigned to cores BY EXPERT (host routes via eids): core c owns
  exactly the tokens that route to expert c, sorted by (batch, position);
  padding duplicates the FIRST (lowest-position) token so pad columns see a
  minimal causal window.  MoE then needs no communication and each core
  loads only its expert.
- Attention: K/V are computed in contiguous position blocks (core r owns
  block r) and shared via one 8-core AllGather pair; after the gather each
  core bulk-loads all K/V into SBUF (8+8 strided DMAs) and runs attention
  with zero per-tile DMA.  All 4 q-heads of a kv group are processed per
  strip so each K/V stationary is loaded once.  Causality is recovered with
  compile-time column windows plus per-core uploaded full-strip masks.
  Softmax runs without max-subtraction (|scores| <= 8).
- The pre-attention rms_norm is algebraically dropped on the Q path (the
  per-head rms_norm cancels any per-token scale); per-head norms are applied
  AFTER rope (scale commutes), with 1/sqrt computed as exp(-0.5*ln) on the
  scalar engine and q_gain folded into the exp bias.
- Layout: all activations transposed [D on partitions, tokens on free].
"""
import contextlib
import numpy as np
import ml_dtypes

import concourse.bass as bass
import concourse.bacc as bacc
import concourse.tile as tile
from concourse import mybir
from concourse.bass_utils import run_bass_kernel_spmd

B, S, D = 2, 2048, 1024
NH, NKV, HD = 16, 4, 64
KVD = NKV * HD
NE, INTER = 8, 512
EPS = float(np.float32(1.1920929e-07))
NCORES = 8
KVBLK = 512          # seq rows per core in the KV phase
NKVT = S // 128      # 16 kv tiles per batch
NDT = D // 128       # 8 d-tiles
F32 = mybir.dt.float32
BF16 = mybir.dt.bfloat16
GRP2 = 1024          # interleaved score-strip group width (2 PSUM banks)
VW = 65              # per-kv-head V width incl. ones column
ALU = mybir.AluOpType
ACT = mybir.ActivationFunctionType
BF = ml_dtypes.bfloat16


# ---------------------------------------------------------------- host side

def _route(eids):
    eids = np.asarray(eids).astype(np.int64)
    lists = [[np.sort(np.where(eids[b] == e)[0]) for b in range(B)]
             for e in range(NE)]
    maxn = max(len(lists[e][b]) for e in range(NE) for b in range(B))
    CB = max(64, ((maxn + 7) // 8) * 8)
    cols = np.zeros((NE, B, CB), dtype=np.int64)
    nreal = np.zeros((NE, B), dtype=np.int64)
    for e in range(NE):
        for b in range(B):
            L = lists[e][b]
            nreal[e, b] = len(L)
            if len(L):
                pad = CB - len(L)
                cols[e, b, pad:] = L
                cols[e, b, :pad] = L[0]
    return cols, nreal, CB


def _windows(cols, CB):
    Wt = np.zeros((B, NKVT), dtype=np.int64)
    for b in range(B):
        for j in range(NKVT):
            Wt[b, j] = min(int(np.searchsorted(cols[e, b], 128 * j))
                           for e in range(NE))
    return Wt


def _strip_groups(Wt, CB):
    """Per batch: stream-pack kv-tile token windows (as head-PAIR blocks of
    2*Nw cols) densely into strip groups of up to GRP2=1024 interleaved
    cols, never letting a block cross a 512-col PSUM bank boundary.
    groups[b] = list of (entries, gwT); entry = (j, Ws, Nw, ofs2)."""
    groups = []
    for b in range(B):
        gs, cur, ofs2 = [], [], 0
        for j in range(NKVT):
            t = int(Wt[b, j])
            while t < CB:
                bank_rem = (512 - (ofs2 % 512)) // 2
                n = min(CB - t, bank_rem)
                cur.append((j, t, n, ofs2))
                ofs2 += 2 * n
                t += n
                if ofs2 == GRP2:
                    gs.append((cur, ofs2 // 2))
                    cur, ofs2 = [], 0
        if cur:
            gs.append((cur, ofs2 // 2))
        groups.append(gs)
    return groups


def _goffs(groups):
    goffs, total = [], 0
    for b in range(B):
        gb = []
        for (g, gwT) in groups[b]:
            gb.append(total)
            total += 2 * gwT
        goffs.append(gb)
    return goffs, max(total, 1)


def _rope_tables(positions):
    """[128, n] cos2/sin2 for full-tile rope (2 heads/tile, swap32 form)."""
    inv_freq = (1.0 / 10000.0 ** (np.arange(0, HD, 2, dtype=np.float32) / HD)
                ).astype(np.float32)
    fr = np.outer(positions.astype(np.float32), inv_freq).astype(np.float32)
    c = np.cos(fr).astype(np.float32).T             # [32, n]
    s = np.sin(fr).astype(np.float32).T
    cos2 = np.concatenate([c, c, c, c], axis=0)
    sin2 = np.concatenate([s, -s, s, -s], axis=0)
    return (np.ascontiguousarray(cos2.astype(BF)),
            np.ascontiguousarray(sin2.astype(BF)))


def _vec8(v, dt=np.float32):
    return np.ascontiguousarray(
        np.asarray(v, np.float32).reshape(NDT, 128).T.astype(dt))


def _build_core_inputs(c, x, x0, vel, rm0, rm1, attn_scale, mlp_scale, mu_c,
                       qg8, cq_wT, ck_wT, cv_wT, proj_wT, gate_up, down,
                       cols, CB, groups, goffs, MW2):
    f = np.float32
    pos = cols[c]                                     # [B, CB]
    bidx = np.repeat(np.arange(B), CB)
    sidx = pos.reshape(-1)
    b_kv, blk = c // 4, c % 4
    rows = slice(KVBLK * blk, KVBLK * blk + KVBLK)
    cosq2, sinq2 = _rope_tables(sidx)
    cosk2, sink2 = _rope_tables(np.arange(KVBLK * blk, KVBLK * blk + KVBLK))
    mask = np.zeros((128, MW2), f)
    for b in range(B):
        for gi, (g, gwT) in enumerate(groups[b]):
            goff = goffs[b][gi]
            for (j, Ws, Nw, ofs2) in g:
                kvp = np.arange(128 * j, 128 * j + 128)
                m = (pos[b, None, Ws:Ws + Nw] >= kvp[:, None])
                # block (de-interleaved) layout: per-head strips
                for hh in range(2):
                    o = goff + hh * gwT + ofs2 // 2
                    mask[:, o:o + Nw] = m
    Tb = lambda a: np.ascontiguousarray(a.T.astype(BF))
    return {
        "xqT": Tb(x[bidx, sidx]), "x0qT": Tb(x0[bidx, sidx]),
        "velqT": Tb(vel[bidx, sidx]),
        "xkvT": Tb(x[b_kv, rows]), "x0kvT": Tb(x0[b_kv, rows]),
        "cq_wT": cq_wT, "ck_wT": ck_wT, "cv_wT": cv_wT, "proj_wT": proj_wT,
        "gu": np.ascontiguousarray(gate_up[c].astype(BF)),
        "dn": np.ascontiguousarray(down[c].astype(BF)),
        "rm0v": _vec8(rm0), "rm1v": _vec8(rm1),
        "ascalev": _vec8(attn_scale), "mscalev": _vec8(mlp_scale),
        "mucv": _vec8(mu_c), "muc3v": _vec8(-0.3 * mu_c),
        "qg8T": np.ascontiguousarray(qg8.reshape(1, NH)),
        "cosq2": cosq2, "sinq2": sinq2, "cosk2": cosk2, "sink2": sink2,
        "mask2": np.ascontiguousarray(mask.astype(BF)),
    }


_PROG_CACHE = {}


def _prep(inputs):
    f = np.float32
    x = np.asarray(inputs["x"], f)
    x0 = np.asarray(inputs["x0"], f)
    vel = np.asarray(inputs["vel"], f)
    resid_mix = np.asarray(inputs["resid_mix"], f)
    mu_c = np.clip(np.asarray(inputs["mu"], f), f(0.5), f(1.5)).astype(f)
    qg8 = (np.asarray(inputs["q_gain"], f) * f(0.125)).astype(f)

    cq_wT = np.ascontiguousarray(np.asarray(inputs["cq_w"], f).T.astype(BF))
    ck_wT = np.ascontiguousarray(np.asarray(inputs["ck_w"], f).T.astype(BF))
    cv_wT = np.ascontiguousarray(np.asarray(inputs["cv_w"], f).T.astype(BF))
    proj_wT = np.ascontiguousarray(np.asarray(inputs["proj_w"], f).T.astype(BF))

    cols, nreal, CB = _route(inputs["eids"])
    Wt = _windows(cols, CB)
    groups = _strip_groups(Wt, CB)
    goffs, MW2 = _goffs(groups)
    meta = (cols, nreal, CB, Wt, groups, goffs, MW2)
    in_maps = [
        _build_core_inputs(c, x, x0, vel, resid_mix[0], resid_mix[1],
                           np.asarray(inputs["attn_scale"], f),
                           np.asarray(inputs["mlp_scale"], f), mu_c, qg8,
                           cq_wT, ck_wT, cv_wT, proj_wT,
                           np.asarray(inputs["gate_up"], f),
                           np.asarray(inputs["down"], f),
                           cols, CB, groups, goffs, MW2)
        for c in range(NCORES)
    ]
    return meta, in_maps


def _assemble(results, meta):
    f = np.float32
    cols, nreal, CB = meta[0], meta[1], meta[2]
    x_out = np.zeros((B, S, D), f)
    v_out = np.zeros((B, S, D), f)
    for c in range(NCORES):
        xoT = results[c]["xoutT"]
        vnT = results[c]["vnT"]
        for b in range(B):
            n = int(nreal[c, b])
            if n == 0:
                continue
            pad = CB - n
            sl = slice(b * CB + pad, b * CB + CB)
            x_out[b, cols[c, b, pad:]] = xoT[:, sl].T
            v_out[b, cols[c, b, pad:]] = vnT[:, sl].T
    return x_out, v_out


def get_program(meta):
    cols, nreal, CB, Wt, groups, goffs, MW2 = meta
    key = (CB, MW2, tuple(Wt.reshape(-1)))
    if key not in _PROG_CACHE:
        _PROG_CACHE[key] = build_program(CB, groups, goffs, MW2)
    return _PROG_CACHE[key]


def kernel(**inputs):
    meta, in_maps = _prep(inputs)
    nc = get_program(meta)
    res = run_bass_kernel_spmd(nc, in_maps, core_ids=list(range(NCORES)))
    return _assemble(res.results, meta)


# ------------------------------------------------------------- device side

def _chunks(n, limit=512):
    return [(s, min(limit, n - s)) for s in range(0, n, limit)]


def build_program(CB, groups, goffs, MW2, n_devices=NCORES, dbg=False):
    C = B * CB
    nc = bacc.Bacc("TRN2", target_bir_lowering=False, debug=False,
                   num_devices=n_devices)
    d_in = {}
    for name, shape in [
        ("xqT", [D, C]), ("x0qT", [D, C]), ("velqT", [D, C]),
        ("xkvT", [D, KVBLK]), ("x0kvT", [D, KVBLK]),
        ("cosq2", [128, C]), ("sinq2", [128, C]),
        ("cosk2", [128, KVBLK]), ("sink2", [128, KVBLK]),
        ("mask2", [128, MW2]),
        ("cq_wT", [D, D]), ("ck_wT", [D, KVD]),
        ("cv_wT", [D, KVD]), ("proj_wT", [D, D]),
        ("gu", [D, 2 * INTER]), ("dn", [INTER, D]),
    ]:
        d_in[name] = nc.dram_tensor(name, shape, BF16, kind="ExternalInput")
    for name, shape in [
        ("rm0v", [128, NDT]), ("rm1v", [128, NDT]),
        ("ascalev", [128, NDT]), ("mscalev", [128, NDT]),
        ("mucv", [128, NDT]), ("muc3v", [128, NDT]),
        ("qg8T", [1, NH]),
    ]:
        d_in[name] = nc.dram_tensor(name, shape, F32, kind="ExternalInput")
    d_xout = nc.dram_tensor("xoutT", [D, C], F32, kind="ExternalOutput")
    d_vn = nc.dram_tensor("vnT", [D, C], F32, kind="ExternalOutput")
    d_dbg = {}
    if dbg:
        for name, shape in [("dbg_qro", [NH // 2 * 64, 2 * C]),
                            ("dbg_agk", [NCORES * KVD, KVBLK]),
                            ("dbg_agv", [NCORES * KVBLK, NKV * VW]),
                            ("dbg_xmq", [D, C]), ("dbg_yall", [D, C]),
                            ("dbg_den", [NH, C])]:
            d_dbg[name] = nc.dram_tensor(name, shape, BF16,
                                         kind="ExternalOutput")

    with tile.TileContext(nc) as tc:
        _emit(tc, nc, d_in, d_xout, d_vn, CB, groups, goffs, d_dbg)
    nc.compile()
    return nc


def _emit(tc, nc, d_in, d_xout, d_vn, CB, groups, goffs, d_dbg={}):
    C = B * CB
    dt = F32
    sy, gp, ve, sc, pe = nc.sync, nc.gpsimd, nc.vector, nc.scalar, nc.tensor

    es = contextlib.ExitStack()
    cst = es.enter_context(tc.tile_pool(name="const", bufs=1))
    agd = es.enter_context(tc.tile_pool(name="agD", bufs=1, space="DRAM"))

    ones128 = cst.tile([128, 1], BF16, tag="ones128")
    ve.memset(ones128[:], 1.0)
    ind65 = cst.tile([128, 65], BF16, tag="ind65")
    ve.memset(ind65[:], 0.0)
    ve.memset(ind65[0:64, 0:1], 1.0)
    ve.memset(ind65[64:128, 64:65], 1.0)
    epsc = cst.tile([128, 1], dt, tag="epsc")
    ve.memset(epsc[:], EPS)
    warm = cst.tile([128, 512], BF16, tag="warm")
    ve.memset(warm[:], 0.001)
    zc65 = cst.tile([1, VW], BF16, tag="zc65")
    ve.memset(zc65[:], 0.0)
    zcb = cst.tile([1, CB], BF16, tag="zcb")
    ve.memset(zcb[:], 0.0)
    vecs = {}
    for nm in ("rm0v", "rm1v", "ascalev", "mscalev", "mucv", "muc3v"):
        t = cst.tile([128, NDT], dt, tag=nm, name=nm)
        sy.dma_start(t[:], d_in[nm].ap())
        vecs[nm] = t
    qg8T = cst.tile([1, NH], dt, tag="qg8T")
    sy.dma_start(qg8T[:], d_in["qg8T"].ap())
    tbl = {}
    for nm, w in (("cosk2", KVBLK), ("sink2", KVBLK)):
        t = cst.tile([128, w], BF16, tag=nm, name=nm)
        sy.dma_start(t[:], d_in[nm].ap())
        tbl[nm] = t

    agk_in = agd.tile([KVD, KVBLK], BF16, tag="agk_in")
    agv_in = agd.tile([KVBLK, NKV * VW], BF16, tag="agv_in")
    agk_out = agd.tile([NCORES * KVD, KVBLK], BF16, addr_space="Shared",
                       tag="agk_out")
    agv_out = agd.tile([NCORES * KVBLK, NKV * VW], BF16, addr_space="Shared",
                       tag="agv_out")

    # PE warmup: keep the HAM clock gate open while input DMAs land.
    with tc.tile_pool(name="wrm", bufs=2, space="PSUM") as wp:
        for _ in range(10):
            t = wp.tile([128, 512], dt, tag="wt", name="wt", bufs=2)
            pe.matmul(t[:], warm[:, 0:128], warm[:], start=True, stop=True)

    def rsqrt_row(rot, src, w, scale, gain, nparts):
        """[nparts, w] PSUM sums -> [nparts, w] f32 gain/sqrt(src*scale+eps)
        via scalar Sqrt + single-pass DVE reciprocal."""
        rt = rot.tile([nparts, w], dt, tag="nrt", name="nrt", bufs=2)
        sc.activation(rt[:], src, ACT.Sqrt, bias=epsc[0:nparts], scale=scale)
        rti = rot.tile([nparts, w], dt, tag="nrti", name="nrti", bufs=2)
        ve.reciprocal_approx_fast(rti[:], rt[:])
        if gain is not None:
            ve.tensor_scalar_mul(rti[:], rti[:], gain)
        return rti

    def rms_norm_T(pool, rot, pstmp, in_tiles, width, out_tag):
        outs = [pool.tile([128, width], BF16, tag=f"{out_tag}{i}",
                          name=f"{out_tag}{i}") for i in range(NDT)]
        for (s, w) in _chunks(width):
            sqs = []
            for i in range(NDT):
                sq = rot.tile([128, w], BF16, tag="nsq", name="nsq", bufs=3)
                sc.activation(sq[:], in_tiles[i][:, s:s + w], ACT.Square)
                sqs.append(sq)
            ssum = pstmp.tile([1, w], dt, tag="nps", name="nps", bufs=2)
            for i in range(NDT):
                pe.matmul(ssum[:], ones128[:], sqs[i][:],
                          start=(i == 0), stop=(i == NDT - 1))
            rti = rsqrt_row(rot, ssum[:], w, 1.0 / D, None, 1)
            rb = rot.tile([1, w], BF16, tag="nrb", name="nrb", bufs=2)
            ve.tensor_copy(rb[:], rti[:])
            bc = rot.tile([128, w], BF16, tag="nbc", name="nbc", bufs=2)
            gp.partition_broadcast(bc[:], rb[0:1, :])
            for i in range(NDT):
                ve.tensor_mul(outs[i][:, s:s + w], in_tiles[i][:, s:s + w],
                              bc[:])
        return outs

    def rope_norm(rot, pstmp, psrc_list, width, cos2, sin2, gb0, gb1,
                  out0, out1, oc0):
        """psrc chunks [128, w] (PSUM, 2 heads pre-norm, pre-rope) ->
        out0/out1 [64, C] slices at col oc0: rope applied, then per-head
        rms_norm scale (folded after rope; q_gain via exp bias)."""
        for ci, (s, w) in enumerate(_chunks(width)):
            psq = psrc_list[ci]
            sq = rot.tile([128, w], BF16, tag="hsq", name="hsq", bufs=2)
            sc.activation(sq[:], psq[:], ACT.Square)
            hs = pstmp.tile([65, w], dt, tag="hps", name="hps", bufs=2)
            pe.matmul(hs[:], ind65[:], sq[:], start=True, stop=True)
            rt = rot.tile([65, w], dt, tag="hrt", name="hrt", bufs=2)
            sc.activation(rt[:], hs[:], ACT.Sqrt, bias=epsc[0:65],
                          scale=1.0 / HD)
            rti0 = rot.tile([1, w], dt, tag="hri0", name="hri0", bufs=2)
            ve.reciprocal_approx_fast(rti0[:], rt[0:1, :])
            if gb0 is not None:
                ve.tensor_scalar_mul(rti0[:], rti0[:], gb0)
            rt1 = rot.tile([1, w], dt, tag="hrt1", name="hrt1", bufs=2)
            ve.tensor_copy(rt1[:], rt[64:65, :])
            rti1 = rot.tile([1, w], dt, tag="hri1", name="hri1", bufs=2)
            ve.reciprocal_approx_fast(rti1[:], rt1[:])
            if gb1 is not None:
                ve.tensor_scalar_mul(rti1[:], rti1[:], gb1)
            sw = rot.tile([128, w], BF16, tag="rsw", name="rsw", bufs=2)
            for base in (0, 64):
                ve.tensor_copy(sw[base:base + 32, :],
                               psq[base + 32:base + 64, :])
                ve.tensor_copy(sw[base + 32:base + 64, :],
                               psq[base:base + 32, :])
            a = rot.tile([128, w], BF16, tag="ra", name="ra", bufs=2)
            ve.tensor_mul(a[:], psq[:], cos2[:, s:s + w])
            ve.tensor_mul(sw[:], sw[:], sin2[:, s:s + w])
            ve.tensor_add(a[:], a[:], sw[:])
            rb0 = rot.tile([1, w], BF16, tag="hrb0", name="hrb0", bufs=2)
            ve.tensor_copy(rb0[:], rti0[:])
            bch = rot.tile([64, w], BF16, tag="hbc", name="hbc", bufs=2)
            gp.partition_broadcast(bch[:], rb0[0:1, :])
            ve.tensor_mul(out0[:, oc0 + s:oc0 + s + w], a[0:64, :], bch[:])
            rb1 = rot.tile([1, w], BF16, tag="hrb1", name="hrb1", bufs=2)
            ve.tensor_copy(rb1[:], rti1[:])
            bch2 = rot.tile([128, w], BF16, tag="hbc2", name="hbc2", bufs=2)
            gp.partition_broadcast(bch2[:], rb1[0:1, :])
            ve.tensor_mul(out1[:, oc0 + s:oc0 + s + w], a[64:128, :],
                          bch2[64:128, :])

    # ============================ Stage A: KV ============================
    with tc.tile_pool(name="kvA", bufs=1) as kva, \
         tc.tile_pool(name="kvR", bufs=2) as kvr, \
         tc.tile_pool(name="kvP", bufs=2, space="PSUM") as kvp:
        xm = []
        for i in range(NDT):
            xk = kvr.tile([128, KVBLK], BF16, tag="xk", name="xk", bufs=2)
            sy.dma_start(xk[:], d_in["xkvT"].ap()[128 * i:128 * (i + 1), :])
            x0k = kvr.tile([128, KVBLK], BF16, tag="x0k", name="x0k", bufs=2)
            sy.dma_start(x0k[:], d_in["x0kvT"].ap()[128 * i:128 * (i + 1), :])
            ve.tensor_scalar_mul(x0k[:], x0k[:], vecs["rm1v"][:, i:i + 1])
            t = kva.tile([128, KVBLK], BF16, tag=f"xmk{i}", name=f"xmk{i}")
            ve.scalar_tensor_tensor(t[:], xk[:], vecs["rm0v"][:, i:i + 1],
                                    x0k[:], ALU.mult, ALU.add)
            xm.append(t)
        nk = rms_norm_T(kva, kvr, kvp, xm, KVBLK, "nk")
        ckw, cvw = [], []
        for i in range(NDT):
            t = kva.tile([128, KVD], BF16, tag=f"ckw{i}", name=f"ckw{i}")
            sy.dma_start(t[:], d_in["ck_wT"].ap()[128 * i:128 * (i + 1), :])
            ckw.append(t)
            t2 = kva.tile([128, KVD], BF16, tag=f"cvw{i}", name=f"cvw{i}")
            sy.dma_start(t2[:], d_in["cv_wT"].ap()[128 * i:128 * (i + 1), :])
            cvw.append(t2)
        kro = kva.tile([128, KVBLK], BF16, tag="kro", name="kro")
        kro2 = kva.tile([128, KVBLK], BF16, tag="kro2", name="kro2")
        for m in range(2):
            pkT = kvp.tile([128, KVBLK], dt, tag="pkT", name="pkT", bufs=2)
            for i in range(NDT):
                pe.matmul(pkT[:], ckw[i][:, 128 * m:128 * (m + 1)], nk[i][:],
                          start=(i == 0), stop=(i == NDT - 1))
            dst = kro if m == 0 else kro2
            rope_norm(kvr, kvp, [pkT], KVBLK, tbl["cosk2"], tbl["sink2"],
                      None, None, dst[0:64, :], dst[64:128, :], 0)
            sy.dma_start(agk_in[128 * m:128 * (m + 1), :], dst[:])
        for m in range(4):
            pv = kvp.tile([128, KVD], dt, tag="pv", name="pv", bufs=2)
            for i in range(NDT):
                pe.matmul(pv[:], nk[i][:, 128 * m:128 * (m + 1)], cvw[i][:],
                          start=(i == 0), stop=(i == NDT - 1))
            vsb = kvr.tile([128, NKV * VW], BF16, tag="vsb", name="vsb",
                           bufs=2)
            for kh in range(NKV):
                ve.tensor_copy(vsb[:, VW * kh:VW * kh + HD],
                               pv[:, HD * kh:HD * (kh + 1)])
                ve.memset(vsb[:, VW * kh + HD:VW * (kh + 1)], 1.0)
            sy.dma_start(agv_in[128 * m:128 * (m + 1), :], vsb[:])

    gp.collective_compute("AllGather", ALU.bypass,
                          replica_groups=[list(range(NCORES))],
                          ins=[agk_in.opt()], outs=[agk_out.opt()])
    gp.collective_compute("AllGather", ALU.bypass,
                          replica_groups=[list(range(NCORES))],
                          ins=[agv_in.opt()], outs=[agv_out.opt()])

    # ===================== Stage B1: Q mix/proj/rope+norm ====================
    qa = es.enter_context(tc.tile_pool(name="qa", bufs=1))      # xmq: ->B4
    yap = es.enter_context(tc.tile_pool(name="yap", bufs=1))    # yall: ->B3
    xmq = [qa.tile([128, C], BF16, tag=f"xmq{i}", name=f"xmq{i}")
           for i in range(NDT)]
    yall = [yap.tile([128, C], BF16, tag=f"yall{i}", name=f"yall{i}")
            for i in range(NDT)]
    qrp = es.enter_context(tc.tile_pool(name="qrop", bufs=1))
    qro2 = [qrp.tile([64, 2 * C], BF16, tag=f"qro{m}", name=f"qro{m}")
            for m in range(NH // 2)]
    with tc.tile_pool(name="qt", bufs=1) as qt, \
         tc.tile_pool(name="qrot", bufs=2) as qr2, \
         tc.tile_pool(name="qP", bufs=2, space="PSUM") as qp:
        for i in range(NDT):
            xq = qr2.tile([128, C], BF16, tag="xq", name="xq", bufs=2)
            sy.dma_start(xq[:], d_in["xqT"].ap()[128 * i:128 * (i + 1), :])
            x0q = qr2.tile([128, C], BF16, tag="x0q", name="x0q", bufs=2)
            sy.dma_start(x0q[:],
                         d_in["x0qT"].ap()[128 * i:128 * (i + 1), :])
            ve.tensor_scalar_mul(x0q[:], x0q[:], vecs["rm1v"][:, i:i + 1])
            ve.scalar_tensor_tensor(xmq[i][:], xq[:],
                                    vecs["rm0v"][:, i:i + 1],
                                    x0q[:], ALU.mult, ALU.add)

        for nm in ("cosq2", "sinq2"):
            t = qt.tile([128, C], BF16, tag=nm, name=nm)
            sy.dma_start(t[:], d_in[nm].ap())
            tbl[nm] = t

        def finish_q(m, psqs):
            qv = qro2[m][:].rearrange("p (f a) -> p f a", a=2)
            rope_norm(qr2, qp, psqs, C, tbl["cosq2"], tbl["sinq2"],
                      qg8T[0:1, 2 * m:2 * m + 1],
                      qg8T[0:1, 2 * m + 1:2 * m + 2],
                      qv[:, :, 0], qv[:, :, 1], 0)

        pending = None
        for half in range(2):
            cqh = []
            for i in range(NDT):
                t = qt.tile([128, 512], BF16, tag=f"cqh{i}",
                            name=f"cqh{i}", bufs=2)
                sy.dma_start(t[:], d_in["cq_wT"].ap()
                             [128 * i:128 * (i + 1),
                              512 * half:512 * (half + 1)])
                cqh.append(t)
            for mm in range(4):
                m = 4 * half + mm
                psqs = []
                for (s, w) in _chunks(C):
                    psq = qp.tile([128, w], dt, tag="psq", name="psq",
                                  bufs=4)
                    for i in range(NDT):
                        pe.matmul(psq[:],
                                  cqh[i][:, 128 * mm:128 * (mm + 1)],
                                  xmq[i][:, s:s + w],
                                  start=(i == 0), stop=(i == NDT - 1))
                    psqs.append(psq)
                if pending is not None:
                    finish_q(*pending)
                pending = (m, psqs)
        finish_q(*pending)

    if d_dbg:
        for m in range(NH // 2):
            sy.dma_start(d_dbg["dbg_qro"].ap()[64 * m:64 * (m + 1), :],
                         qro2[m][:])
        for i in range(NDT):
            sy.dma_start(d_dbg["dbg_xmq"].ap()[128 * i:128 * (i + 1), :],
                         xmq[i][:])
        sy.dma_start(d_dbg["dbg_agk"].ap(), agk_out[:, :])
        sy.dma_start(d_dbg["dbg_agv"].ap(), agv_out[:, :])

    # ================== K/V bulk preload into SBUF ==================
    atk = es.enter_context(tc.tile_pool(name="atk", bufs=1))
    mask_sb = atk.tile([128, d_in["mask2"].shape[1]], BF16, tag="mask")
    kk, vv = [], []
    for r in range(NCORES):
        kt = atk.tile([64, NKV * KVBLK], BF16, tag=f"kk{r}", name=f"kk{r}")
        src = agk_out[KVD * r:KVD * (r + 1), :].rearrange(
            "(a p) f -> p a f", a=NKV)
        sy.dma_start(kt[:].rearrange("p (a f) -> p a f", a=NKV), src)
        kk.append(kt)
        vt = atk.tile([128, 4 * NKV * VW], BF16, tag=f"vv{r}", name=f"vv{r}")
        srcv = agv_out[KVBLK * r:KVBLK * (r + 1), :].rearrange(
            "(a p) f -> p a f", a=4)
        sy.dma_start(vt[:].rearrange("p (a f) -> p a f", a=4), srcv)
        vv.append(vt)
    sy.dma_start(mask_sb[:], d_in["mask2"].ap())

    # ========================= Stage B2: attention =========================
    # Per (b, kv-head, head-pair): both heads of the pair are computed in a
    # SINGLE score matmul per kv-tile entry (token-interleaved layout), then
    # exp (one call per strip group), then one mask-multiply that also
    # de-interleaves into per-head blocks for the V matmuls.  Groups are
    # software-pipelined so exp/mask of group g overlaps scores of g+1.
    with tc.tile_pool(name="at", bufs=1) as at, \
         tc.tile_pool(name="atP", bufs=2, space="PSUM") as atp:
        for b in range(B):
            last_j = groups[b][-1][0][-1][0]
            for kh in range(NKV):
                for hp in range(2):
                    m = 2 * kh + hp
                    h0 = 2 * m

                    def emit_v(gd, pys):
                        g, gwT, goff, sts, prb2 = gd
                        # exp de-interleaves: in (f a) token-interleaved,
                        # out (a f) per-head blocks
                        si = sts[:, 0:2 * gwT].rearrange(
                            "p (f a) -> p a f", a=2)
                        po = prb2[:, 0:2 * gwT].rearrange(
                            "p (a f) -> p a f", a=2)
                        sc.activation(po, si, ACT.Exp)
                        ve.tensor_mul(prb2[:, 0:2 * gwT], prb2[:, 0:2 * gwT],
                                      mask_sb[:, goff:goff + 2 * gwT])
                        for (j, Ws, Nw, ofs2) in g:
                            r = 4 * b + j // 4
                            vxj = vv[r][:, NKV * VW * (j % 4) + VW * kh:
                                        NKV * VW * (j % 4) + VW * (kh + 1)]
                            for hh in range(2):
                                pe.matmul(
                                    pys[hh][:, Ws:Ws + Nw], vxj,
                                    prb2[:, gwT * hh + ofs2 // 2:
                                         gwT * hh + ofs2 // 2 + Nw],
                                    start=False, stop=(j == last_j),
                                    skip_group_check=True)

                    pys = [atp.tile([VW, CB], dt, tag="py", name="py",
                                    bufs=2) for _ in range(2)]
                    for hh in range(2):
                        pe.matmul(pys[hh][:, 0:CB], zc65[:], zcb[:],
                                  start=True, stop=False,
                                  skip_group_check=True)
                    pend = None
                    for gi, (g, gwT) in enumerate(groups[b]):
                        goff = goffs[b][gi]
                        sts = atp.tile([128, 2 * gwT], dt, tag="st",
                                       name="st", bufs=3,
                                       padded_shape=[128, GRP2])
                        prb2 = at.tile([128, 2 * gwT], BF16, tag="prb2",
                                       name="prb2", bufs=3,
                                       padded_shape=[128, GRP2])
                        for (j, Ws, Nw, ofs2) in g:
                            r = 4 * b + j // 4
                            loc = KVBLK * kh + 128 * (j % 4)
                            ktj = kk[r][:, loc:loc + 128]
                            qs = qro2[m][:, 2 * (b * CB + Ws):
                                         2 * (b * CB + Ws + Nw)]
                            pe.matmul(sts[:, ofs2:ofs2 + 2 * Nw],
                                      ktj, qs, start=True, stop=True)
                        if pend is not None:
                            emit_v(pend, pys)
                        pend = (g, gwT, goff, sts, prb2)
                    emit_v(pend, pys)
                    for hh in range(2):
                        h = h0 + hh
                        rc0 = at.tile([1, CB], dt, tag="rc0", name="rc0",
                                      bufs=2)
                        ve.tensor_copy(rc0[:], pys[hh][64:65, :])
                        if d_dbg:
                            dce = at.tile([1, CB], BF16, tag="dce",
                                          name="dce", bufs=2)
                            ve.tensor_copy(dce[:], rc0[:])
                            sy.dma_start(
                                d_dbg["dbg_den"].ap()[h:h + 1,
                                                      b * CB:b * CB + CB],
                                dce[:])
                        rc = at.tile([1, CB], dt, tag="rc", name="rc",
                                     bufs=2)
                        ve.reciprocal_approx_fast(rc[:], rc0[:])
                        yb = at.tile([64, CB], dt, tag="yb", name="yb",
                                     bufs=2)
                        gp.partition_broadcast(yb[:], rc[0:1, :])
                        ve.tensor_mul(
                            yall[h // 2][64 * (h % 2):64 * (h % 2) + 64,
                                         b * CB:b * CB + CB],
                            pys[hh][0:64, :], yb[:])

    if d_dbg:
        for i in range(NDT):
            sy.dma_start(d_dbg["dbg_yall"].ap()[128 * i:128 * (i + 1), :],
                         yall[i][:])

    # ===================== Stage B3: out-proj + PID =====================
    with tc.tile_pool(name="pj", bufs=1) as pj, \
         tc.tile_pool(name="pjR", bufs=2) as pjr, \
         tc.tile_pool(name="pjP", bufs=2, space="PSUM") as pjp:
        for half in range(2):
            pjh = []
            for i in range(NDT):
                t = pj.tile([128, 512], BF16, tag=f"pjh{i}", name=f"pjh{i}",
                            bufs=2)
                sy.dma_start(t[:], d_in["proj_wT"].ap()
                             [128 * i:128 * (i + 1),
                              512 * half:512 * (half + 1)])
                pjh.append(t)
            for mm in range(4):
                m = 4 * half + mm
                velm = pjr.tile([128, C], BF16, tag="velm", name="velm",
                                bufs=2)
                sy.dma_start(velm[:],
                             d_in["velqT"].ap()[128 * m:128 * (m + 1), :])
                for (s, w) in _chunks(C):
                    pso = pjp.tile([128, w], dt, tag="pso", name="pso",
                                   bufs=2)
                    for i in range(NDT):
                        pe.matmul(pso[:], pjh[i][:, 128 * mm:128 * (mm + 1)],
                                  yall[i][:, s:s + w],
                                  start=(i == 0), stop=(i == NDT - 1))
                    ve.scalar_tensor_tensor(
                        xmq[m][:, s:s + w], pso[:],
                        vecs["ascalev"][:, m:m + 1],
                        xmq[m][:, s:s + w], ALU.mult, ALU.add)
                t2 = pjr.tile([128, C], dt, tag="t2", name="t2", bufs=2)
                sc.activation(t2[:], xmq[m][:], ACT.Identity,
                              bias=vecs["muc3v"][:, m:m + 1], scale=0.3)
                vn = pjr.tile([128, C], dt, tag="vn", name="vn", bufs=2)
                ve.scalar_tensor_tensor(vn[:], velm[:], 0.95, t2[:],
                                        ALU.mult, ALU.subtract)
                ve.tensor_scalar(vn[:], vn[:], 3.0, -3.0, ALU.min, ALU.max)
                sy.dma_start(d_vn.ap()[128 * m:128 * (m + 1), :], vn[:])
                ve.scalar_tensor_tensor(xmq[m][:], vn[:], 0.1 * 0.1,
                                        xmq[m][:], ALU.mult, ALU.add)

    # ============================ Stage B4: MoE ============================
    with tc.tile_pool(name="mo", bufs=1) as mo, \
         tc.tile_pool(name="moR", bufs=2) as mor, \
         tc.tile_pool(name="moP", bufs=2, space="PSUM") as mop:
        mn = rms_norm_T(mo, mor, mop, xmq, C, "mn")
        sg, hh_t = [], []
        for half in range(2):
            guh = []
            for i in range(NDT):
                t = mo.tile([128, 512], BF16, tag=f"guh{i}", name=f"guh{i}",
                            bufs=2)
                sy.dma_start(t[:], d_in["gu"].ap()
                             [128 * i:128 * (i + 1),
                              512 * half:512 * (half + 1)])
                guh.append(t)
            for mm in range(4):
                m = 4 * half + mm
                for (s, w) in _chunks(C):
                    psh = mop.tile([128, w], dt, tag="psh", name="psh",
                                   bufs=3)
                    for i in range(NDT):
                        pe.matmul(psh[:], guh[i][:, 128 * mm:128 * (mm + 1)],
                                  mn[i][:, s:s + w],
                                  start=(i == 0), stop=(i == NDT - 1))
                    if m < 4:
                        if s == 0:
                            sgm = mo.tile([128, C], dt, tag=f"sg{m}",
                                          name=f"sg{m}")
                            sg.append(sgm)
                        sc.activation(sg[m][:, s:s + w], psh[:], ACT.Silu)
                    else:
                        if s == 0:
                            hm = mo.tile([128, C], BF16, tag=f"hh{m - 4}",
                                         name=f"hh{m - 4}")
                            hh_t.append(hm)
                        ve.tensor_mul(hh_t[m - 4][:, s:s + w],
                                      sg[m - 4][:, s:s + w], psh[:])
        dnw = []
        for i2 in range(4):
            t = mo.tile([128, D], BF16, tag=f"dnw{i2}", name=f"dnw{i2}")
            sy.dma_start(t[:], d_in["dn"].ap()[128 * i2:128 * (i2 + 1), :])
            dnw.append(t)
        for m in range(NDT):
            xo = mor.tile([128, C], dt, tag="xo", name="xo", bufs=2)
            for (s, w) in _chunks(C):
                psm = mop.tile([128, w], dt, tag="psm", name="psm", bufs=2)
                for i2 in range(4):
                    pe.matmul(psm[:], dnw[i2][:, 128 * m:128 * (m + 1)],
                              hh_t[i2][:, s:s + w],
                              start=(i2 == 0), stop=(i2 == 3))
                ve.scalar_tensor_tensor(xo[:, s:s + w], psm[:],
                                        vecs["mscalev"][:, m:m + 1],
                                        xmq[m][:, s:s + w],
                                        ALU.mult, ALU.add)
            sy.dma_start(d_xout.ap()[128 * m:128 * (m + 1), :], xo[:])

    es.close()


# revision 74
# speedup vs baseline: 1.0474x; 1.0474x over previous
"""Trainium2 Bass kernel for nn_Block_85598698209846 (moe_routing).

Strategy (8 NeuronCores, SPMD single program, per-core data):
- Tokens are assigned to cores BY EXPERT (host routes via eids): core c owns
  exactly the tokens that route to expert c, sorted by (batch, position);
  padding duplicates the FIRST (lowest-position) token so pad columns see a
  minimal causal window.  MoE then needs no communication and each core
  loads only its expert.
- Attention: K/V are computed in contiguous position blocks (core r owns
  block r) and shared via one 8-core AllGather pair; after the gather each
  core bulk-loads all K/V into SBUF (8+8 strided DMAs) and runs attention
  with zero per-tile DMA.  All 4 q-heads of a kv group are processed per
  strip so each K/V stationary is loaded once.  Causality is recovered with
  compile-time column windows plus per-core uploaded full-strip masks.
  Softmax runs without max-subtraction (|scores| <= 8).
- The pre-attention rms_norm is algebraically dropped on the Q path (the
  per-head rms_norm cancels any per-token scale); per-head norms are applied
  AFTER rope (scale commutes), with 1/sqrt computed as exp(-0.5*ln) on the
  scalar engine and q_gain folded into the exp bias.
- Layout: all activations transposed [D on partitions, tokens on free].
"""
import contextlib
import numpy as np
import ml_dtypes

import concourse.bass as bass
import concourse.bacc as bacc
import concourse.tile as tile
from concourse import mybir
from concourse.bass_utils import run_bass_kernel_spmd

B, S, D = 2, 2048, 1024
NH, NKV, HD = 16, 4, 64
KVD = NKV * HD
NE, INTER = 8, 512
EPS = float(np.float32(1.1920929e-07))
NCORES = 8
KVBLK = 512          # seq rows per core in the KV phase
NKVT = S // 128      # 16 kv tiles per batch
NDT = D // 128       # 8 d-tiles
F32 = mybir.dt.float32
BF16 = mybir.dt.bfloat16
GRP2 = 1024          # interleaved score-strip group width (2 PSUM banks)
VW = 65              # per-kv-head V width incl. ones column
ALU = mybir.AluOpType
ACT = mybir.ActivationFunctionType
BF = ml_dtypes.bfloat16


# ---------------------------------------------------------------- host side

def _route(eids):
    eids = np.asarray(eids).astype(np.int64)
    lists = [[np.sort(np.where(eids[b] == e)[0]) for b in range(B)]
             for e in range(NE)]
    maxn = max(len(lists[e][b]) for e in range(NE) for b in range(B))
    CB = max(64, ((maxn + 7) // 8) * 8)
    cols = np.zeros((NE, B, CB), dtype=np.int64)
    nreal = np.zeros((NE, B), dtype=np.int64)
    for e in range(NE):
        for b in range(B):
            L = lists[e][b]
            nreal[e, b] = len(L)
            if len(L):
                pad = CB - len(L)
                cols[e, b, pad:] = L
                cols[e, b, :pad] = L[0]
    return cols, nreal, CB


def _windows(cols, CB):
    Wt = np.zeros((B, NKVT), dtype=np.int64)
    for b in range(B):
        for j in range(NKVT):
            Wt[b, j] = min(int(np.searchsorted(cols[e, b], 128 * j))
                           for e in range(NE))
    return Wt


def _strip_groups(Wt, CB):
    """Per batch: stream-pack kv-tile token windows (as head-PAIR blocks of
    2*Nw cols) densely into strip groups of up to GRP2=1024 interleaved
    cols, never letting a block cross a 512-col PSUM bank boundary.
    groups[b] = list of (entries, gwT); entry = (j, Ws, Nw, ofs2)."""
    groups = []
    for b in range(B):
        gs, cur, ofs2 = [], [], 0
        for j in range(NKVT):
            t = int(Wt[b, j])
            while t < CB:
                bank_rem = (512 - (ofs2 % 512)) // 2
                n = min(CB - t, bank_rem)
                cur.append((j, t, n, ofs2))
                ofs2 += 2 * n
                t += n
                if ofs2 == GRP2:
                    gs.append((cur, ofs2 // 2))
                    cur, ofs2 = [], 0
        if cur:
            gs.append((cur, ofs2 // 2))
        groups.append(gs)
    return groups


def _goffs(groups):
    goffs, total = [], 0
    for b in range(B):
        gb = []
        for (g, gwT) in groups[b]:
            gb.append(total)
            total += 2 * gwT
        goffs.append(gb)
    return goffs, max(total, 1)


def _rope_tables(positions):
    """[128, n] cos2/sin2 for full-tile rope (2 heads/tile, swap32 form)."""
    inv_freq = (1.0 / 10000.0 ** (np.arange(0, HD, 2, dtype=np.float32) / HD)
                ).astype(np.float32)
    fr = np.outer(positions.astype(np.float32), inv_freq).astype(np.float32)
    c = np.cos(fr).astype(np.float32).T             # [32, n]
    s = np.sin(fr).astype(np.float32).T
    cos2 = np.concatenate([c, c, c, c], axis=0)
    sin2 = np.concatenate([s, -s, s, -s], axis=0)
    return (np.ascontiguousarray(cos2.astype(BF)),
            np.ascontiguousarray(sin2.astype(BF)))


def _vec8(v, dt=np.float32):
    return np.ascontiguousarray(
        np.asarray(v, np.float32).reshape(NDT, 128).T.astype(dt))


def _build_core_inputs(c, x, x0, vel, rm0, rm1, attn_scale, mlp_scale, mu_c,
                       qg8, cq_wT, ck_wT, cv_wT, proj_wT, gate_up, down,
                       cols, CB, groups, goffs, MW2):
    f = np.float32
    pos = cols[c]                                     # [B, CB]
    bidx = np.repeat(np.arange(B), CB)
    sidx = pos.reshape(-1)
    b_kv, blk = c // 4, c % 4
    rows = slice(KVBLK * blk, KVBLK * blk + KVBLK)
    cosq2, sinq2 = _rope_tables(sidx)
    cosk2, sink2 = _rope_tables(np.arange(KVBLK * blk, KVBLK * blk + KVBLK))
    mask = np.zeros((128, MW2), f)
    for b in range(B):
        for gi, (g, gwT) in enumerate(groups[b]):
            goff = goffs[b][gi]
            for (j, Ws, Nw, ofs2) in g:
                kvp = np.arange(128 * j, 128 * j + 128)
                m = (pos[b, None, Ws:Ws + Nw] >= kvp[:, None])
                # block (de-interleaved) layout: per-head strips
                for hh in range(2):
                    o = goff + hh * gwT + ofs2 // 2
                    mask[:, o:o + Nw] = m
    Tb = lambda a: np.ascontiguousarray(a.T.astype(BF))
    return {
        "xqT": Tb(x[bidx, sidx]), "x0qT": Tb(x0[bidx, sidx]),
        "velqT": Tb(vel[bidx, sidx]),
        "xkvT": Tb(x[b_kv, rows]), "x0kvT": Tb(x0[b_kv, rows]),
        "cq_wT": cq_wT, "ck_wT": ck_wT, "cv_wT": cv_wT, "proj_wT": proj_wT,
        "gu": np.ascontiguousarray(gate_up[c].astype(BF)),
        "dn": np.ascontiguousarray(down[c].astype(BF)),
        "rm0v": _vec8(rm0), "rm1v": _vec8(rm1),
        "ascalev": _vec8(attn_scale), "mscalev": _vec8(mlp_scale),
        "mucv": _vec8(mu_c), "muc3v": _vec8(-0.3 * mu_c),
        "qg8T": np.ascontiguousarray(qg8.reshape(1, NH)),
        "qg65": np.ascontiguousarray(
            np.concatenate([qg8[0::2].reshape(1, NH // 2),
                            np.ones((63, NH // 2), f),
                            qg8[1::2].reshape(1, NH // 2)], axis=0)),
        "cosq2": cosq2, "sinq2": sinq2, "cosk2": cosk2, "sink2": sink2,
        "mask2": np.ascontiguousarray(mask.astype(BF)),
    }


_PROG_CACHE = {}


def _prep(inputs):
    f = np.float32
    x = np.asarray(inputs["x"], f)
    x0 = np.asarray(inputs["x0"], f)
    vel = np.asarray(inputs["vel"], f)
    resid_mix = np.asarray(inputs["resid_mix"], f)
    mu_c = np.clip(np.asarray(inputs["mu"], f), f(0.5), f(1.5)).astype(f)
    qg8 = (np.asarray(inputs["q_gain"], f) * f(0.125)).astype(f)

    cq_wT = np.ascontiguousarray(np.asarray(inputs["cq_w"], f).T.astype(BF))
    ck_wT = np.ascontiguousarray(np.asarray(inputs["ck_w"], f).T.astype(BF))
    cv_wT = np.ascontiguousarray(np.asarray(inputs["cv_w"], f).T.astype(BF))
    proj_wT = np.ascontiguousarray(np.asarray(inputs["proj_w"], f).T.astype(BF))

    cols, nreal, CB = _route(inputs["eids"])
    Wt = _windows(cols, CB)
    groups = _strip_groups(Wt, CB)
    goffs, MW2 = _goffs(groups)
    meta = (cols, nreal, CB, Wt, groups, goffs, MW2)
    in_maps = [
        _build_core_inputs(c, x, x0, vel, resid_mix[0], resid_mix[1],
                           np.asarray(inputs["attn_scale"], f),
                           np.asarray(inputs["mlp_scale"], f), mu_c, qg8,
                           cq_wT, ck_wT, cv_wT, proj_wT,
                           np.asarray(inputs["gate_up"], f),
                           np.asarray(inputs["down"], f),
                           cols, CB, groups, goffs, MW2)
        for c in range(NCORES)
    ]
    return meta, in_maps


def _assemble(results, meta):
    f = np.float32
    cols, nreal, CB = meta[0], meta[1], meta[2]
    x_out = np.zeros((B, S, D), f)
    v_out = np.zeros((B, S, D), f)
    for c in range(NCORES):
        xoT = results[c]["xoutT"]
        vnT = results[c]["vnT"]
        for b in range(B):
            n = int(nreal[c, b])
            if n == 0:
                continue
            pad = CB - n
            sl = slice(b * CB + pad, b * CB + CB)
            x_out[b, cols[c, b, pad:]] = xoT[:, sl].T
            v_out[b, cols[c, b, pad:]] = vnT[:, sl].T
    return x_out, v_out


def get_program(meta):
    cols, nreal, CB, Wt, groups, goffs, MW2 = meta
    key = (CB, MW2, tuple(Wt.reshape(-1)))
    if key not in _PROG_CACHE:
        _PROG_CACHE[key] = build_program(CB, groups, goffs, MW2)
    return _PROG_CACHE[key]


def kernel(**inputs):
    meta, in_maps = _prep(inputs)
    nc = get_program(meta)
    res = run_bass_kernel_spmd(nc, in_maps, core_ids=list(range(NCORES)))
    return _assemble(res.results, meta)


# ------------------------------------------------------------- device side

def _chunks(n, limit=512):
    return [(s, min(limit, n - s)) for s in range(0, n, limit)]


def build_program(CB, groups, goffs, MW2, n_devices=NCORES, dbg=False):
    C = B * CB
    nc = bacc.Bacc("TRN2", target_bir_lowering=False, debug=False,
                   num_devices=n_devices)
    d_in = {}
    for name, shape in [
        ("xqT", [D, C]), ("x0qT", [D, C]), ("velqT", [D, C]),
        ("xkvT", [D, KVBLK]), ("x0kvT", [D, KVBLK]),
        ("cosq2", [128, C]), ("sinq2", [128, C]),
        ("cosk2", [128, KVBLK]), ("sink2", [128, KVBLK]),
        ("mask2", [128, MW2]),
        ("cq_wT", [D, D]), ("ck_wT", [D, KVD]),
        ("cv_wT", [D, KVD]), ("proj_wT", [D, D]),
        ("gu", [D, 2 * INTER]), ("dn", [INTER, D]),
    ]:
        d_in[name] = nc.dram_tensor(name, shape, BF16, kind="ExternalInput")
    for name, shape in [
        ("rm0v", [128, NDT]), ("rm1v", [128, NDT]),
        ("ascalev", [128, NDT]), ("mscalev", [128, NDT]),
        ("mucv", [128, NDT]), ("muc3v", [128, NDT]),
        ("qg8T", [1, NH]), ("qg65", [VW, NH // 2]),
    ]:
        d_in[name] = nc.dram_tensor(name, shape, F32, kind="ExternalInput")
    d_xout = nc.dram_tensor("xoutT", [D, C], F32, kind="ExternalOutput")
    d_vn = nc.dram_tensor("vnT", [D, C], F32, kind="ExternalOutput")
    d_dbg = {}
    if dbg:
        for name, shape in [("dbg_qro", [NH // 2 * 64, 2 * C]),
                            ("dbg_agk", [NCORES * KVD, KVBLK]),
                            ("dbg_agv", [NCORES * KVBLK, NKV * VW]),
                            ("dbg_xmq", [D, C]), ("dbg_yall", [D, C]),
                            ("dbg_den", [NH, C])]:
            d_dbg[name] = nc.dram_tensor(name, shape, BF16,
                                         kind="ExternalOutput")

    with tile.TileContext(nc) as tc:
        _emit(tc, nc, d_in, d_xout, d_vn, CB, groups, goffs, d_dbg)
    nc.compile()
    return nc


def _emit(tc, nc, d_in, d_xout, d_vn, CB, groups, goffs, d_dbg={}):
    C = B * CB
    dt = F32
    sy, gp, ve, sc, pe = nc.sync, nc.gpsimd, nc.vector, nc.scalar, nc.tensor

    es = contextlib.ExitStack()
    cst = es.enter_context(tc.tile_pool(name="const", bufs=1))
    agd = es.enter_context(tc.tile_pool(name="agD", bufs=1, space="DRAM"))

    ones128 = cst.tile([128, 1], BF16, tag="ones128")
    ve.memset(ones128[:], 1.0)
    ind65 = cst.tile([128, 65], BF16, tag="ind65")
    ve.memset(ind65[:], 0.0)
    ve.memset(ind65[0:64, 0:1], 1.0)
    ve.memset(ind65[64:128, 64:65], 1.0)
    epsc = cst.tile([128, 1], dt, tag="epsc")
    ve.memset(epsc[:], EPS)
    warm = cst.tile([128, 512], BF16, tag="warm")
    ve.memset(warm[:], 0.001)
    zc65 = cst.tile([1, VW], BF16, tag="zc65")
    ve.memset(zc65[:], 0.0)
    zcb = cst.tile([1, CB], BF16, tag="zcb")
    ve.memset(zcb[:], 0.0)
    vecs = {}
    for nm in ("rm0v", "rm1v", "ascalev", "mscalev", "mucv", "muc3v"):
        t = cst.tile([128, NDT], dt, tag=nm, name=nm)
        sy.dma_start(t[:], d_in[nm].ap())
        vecs[nm] = t
    qg8T = cst.tile([1, NH], dt, tag="qg8T")
    sy.dma_start(qg8T[:], d_in["qg8T"].ap())
    qg65 = cst.tile([VW, NH // 2], dt, tag="qg65")
    sy.dma_start(qg65[:], d_in["qg65"].ap())
    tbl = {}
    for nm, w in (("cosk2", KVBLK), ("sink2", KVBLK)):
        t = cst.tile([128, w], BF16, tag=nm, name=nm)
        sy.dma_start(t[:], d_in[nm].ap())
        tbl[nm] = t

    agk_in = agd.tile([KVD, KVBLK], BF16, tag="agk_in")
    agv_in = agd.tile([KVBLK, NKV * VW], BF16, tag="agv_in")
    agk_out = agd.tile([NCORES * KVD, KVBLK], BF16, addr_space="Shared",
                       tag="agk_out")
    agv_out = agd.tile([NCORES * KVBLK, NKV * VW], BF16, addr_space="Shared",
                       tag="agv_out")

    # PE warmup: keep the HAM clock gate open while input DMAs land.
    with tc.tile_pool(name="wrm", bufs=2, space="PSUM") as wp:
        for _ in range(10):
            t = wp.tile([128, 512], dt, tag="wt", name="wt", bufs=2)
            pe.matmul(t[:], warm[:, 0:128], warm[:], start=True, stop=True)

    def rsqrt_row(rot, src, w, scale, gain, nparts):
        """[nparts, w] PSUM sums -> [nparts, w] f32 gain/sqrt(src*scale+eps)
        via scalar Sqrt + single-pass DVE reciprocal."""
        rt = rot.tile([nparts, w], dt, tag="nrt", name="nrt", bufs=2)
        sc.activation(rt[:], src, ACT.Sqrt, bias=epsc[0:nparts], scale=scale)
        rti = rot.tile([nparts, w], dt, tag="nrti", name="nrti", bufs=2)
        ve.reciprocal_approx_fast(rti[:], rt[:])
        if gain is not None:
            ve.tensor_scalar_mul(rti[:], rti[:], gain)
        return rti

    def rms_norm_T(pool, rot, pstmp, in_tiles, width, out_tag):
        outs = [pool.tile([128, width], BF16, tag=f"{out_tag}{i}",
                          name=f"{out_tag}{i}") for i in range(NDT)]
        for (s, w) in _chunks(width):
            sqs = []
            for i in range(NDT):
                sq = rot.tile([128, w], BF16, tag="nsq", name="nsq", bufs=3)
                sc.activation(sq[:], in_tiles[i][:, s:s + w], ACT.Square)
                sqs.append(sq)
            ssum = pstmp.tile([1, w], dt, tag="nps", name="nps", bufs=2)
            for i in range(NDT):
                pe.matmul(ssum[:], ones128[:], sqs[i][:],
                          start=(i == 0), stop=(i == NDT - 1))
            rti = rsqrt_row(rot, ssum[:], w, 1.0 / D, None, 1)
            rb = rot.tile([1, w], BF16, tag="nrb", name="nrb", bufs=2)
            ve.tensor_copy(rb[:], rti[:])
            bc = rot.tile([128, w], BF16, tag="nbc", name="nbc", bufs=2)
            gp.partition_broadcast(bc[:], rb[0:1, :])
            for i in range(NDT):
                ve.tensor_mul(outs[i][:, s:s + w], in_tiles[i][:, s:s + w],
                              bc[:])
        return outs

    def rope_norm(rot, pstmp, psrc_list, width, cos2, sin2, gcol,
                  out0, out1, oc0):
        """psrc chunks [128, w] (PSUM, 2 heads pre-norm, pre-rope) ->
        out0/out1 [64, C] slices at col oc0: rope applied, then per-head
        rms_norm scale (folded after rope; q_gain via exp bias)."""
        for ci, (s, w) in enumerate(_chunks(width)):
            psq = psrc_list[ci]
            sq = rot.tile([128, w], BF16, tag="hsq", name="hsq", bufs=2)
            sc.activation(sq[:], psq[:], ACT.Square)
            hs = pstmp.tile([65, w], dt, tag="hps", name="hps", bufs=2)
            pe.matmul(hs[:], ind65[:], sq[:], start=True, stop=True)
            rt = rot.tile([65, w], dt, tag="hrt", name="hrt", bufs=2)
            sc.activation(rt[:], hs[:], ACT.Sqrt, bias=epsc[0:65],
                          scale=1.0 / HD)
            rti = rot.tile([65, w], dt, tag="hri", name="hri", bufs=2)
            ve.reciprocal_approx_fast(rti[:], rt[:])
            if gcol is not None:
                ve.tensor_scalar_mul(rti[:], rti[:], gcol)
            rb = rot.tile([65, w], BF16, tag="hrb", name="hrb", bufs=2)
            ve.tensor_copy(rb[:], rti[:])
            pq = rot.tile([128, w], BF16, tag="rpq", name="rpq", bufs=2)
            ve.tensor_copy(pq[:], psq[:])
            sw = rot.tile([128, w], BF16, tag="rsw", name="rsw", bufs=2)
            for base in (0, 64):
                ve.tensor_copy(sw[base:base + 32, :],
                               pq[base + 32:base + 64, :])
                ve.tensor_copy(sw[base + 32:base + 64, :],
                               pq[base:base + 32, :])
            a = rot.tile([128, w], BF16, tag="ra", name="ra", bufs=2)
            ve.tensor_mul(a[:], pq[:], cos2[:, s:s + w])
            ve.tensor_mul(sw[:], sw[:], sin2[:, s:s + w])
            ve.tensor_add(a[:], a[:], sw[:])
            bch = rot.tile([64, w], BF16, tag="hbc", name="hbc", bufs=2)
            gp.partition_broadcast(bch[:], rb[0:1, :])
            ve.tensor_mul(out0[:, oc0 + s:oc0 + s + w], a[0:64, :], bch[:])
            rb1 = rot.tile([1, w], BF16, tag="hrb1", name="hrb1", bufs=2)
            ve.tensor_copy(rb1[:], rb[64:65, :])
            bch2 = rot.tile([128, w], BF16, tag="hbc2", name="hbc2", bufs=2)
            gp.partition_broadcast(bch2[:], rb1[0:1, :])
            ve.tensor_mul(out1[:, oc0 + s:oc0 + s + w], a[64:128, :],
                          bch2[64:128, :])

    # ============================ Stage A: KV ============================
    with tc.tile_pool(name="kvA", bufs=1) as kva, \
         tc.tile_pool(name="kvR", bufs=2) as kvr, \
         tc.tile_pool(name="kvP", bufs=2, space="PSUM") as kvp:
        xm = []
        for i in range(NDT):
            xk = kvr.tile([128, KVBLK], BF16, tag="xk", name="xk", bufs=2)
            sc.dma_start(xk[:], d_in["xkvT"].ap()[128 * i:128 * (i + 1), :])
            x0k = kvr.tile([128, KVBLK], BF16, tag="x0k", name="x0k", bufs=2)
            sc.dma_start(x0k[:], d_in["x0kvT"].ap()[128 * i:128 * (i + 1), :])
            ve.tensor_scalar_mul(x0k[:], x0k[:], vecs["rm1v"][:, i:i + 1])
            t = kva.tile([128, KVBLK], BF16, tag=f"xmk{i}", name=f"xmk{i}")
            ve.scalar_tensor_tensor(t[:], xk[:], vecs["rm0v"][:, i:i + 1],
                                    x0k[:], ALU.mult, ALU.add)
            xm.append(t)
        nk = rms_norm_T(kva, kvr, kvp, xm, KVBLK, "nk")
        ckw, cvw = [], []
        for i in range(NDT):
            t = kva.tile([128, KVD], BF16, tag=f"ckw{i}", name=f"ckw{i}")
            sc.dma_start(t[:], d_in["ck_wT"].ap()[128 * i:128 * (i + 1), :])
            ckw.append(t)
            t2 = kva.tile([128, KVD], BF16, tag=f"cvw{i}", name=f"cvw{i}")
            sc.dma_start(t2[:], d_in["cv_wT"].ap()[128 * i:128 * (i + 1), :])
            cvw.append(t2)
        kro = kva.tile([128, KVBLK], BF16, tag="kro", name="kro")
        kro2 = kva.tile([128, KVBLK], BF16, tag="kro2", name="kro2")
        for m in range(2):
            pkT = kvp.tile([128, KVBLK], dt, tag="pkT", name="pkT", bufs=2)
            for i in range(NDT):
                pe.matmul(pkT[:], ckw[i][:, 128 * m:128 * (m + 1)], nk[i][:],
                          start=(i == 0), stop=(i == NDT - 1))
            dst = kro if m == 0 else kro2
            rope_norm(kvr, kvp, [pkT], KVBLK, tbl["cosk2"], tbl["sink2"],
                      None, dst[0:64, :], dst[64:128, :], 0)
            sy.dma_start(agk_in[128 * m:128 * (m + 1), :], dst[:])
        for m in range(4):
            pv = kvp.tile([128, KVD], dt, tag="pv", name="pv", bufs=2)
            for i in range(NDT):
                pe.matmul(pv[:], nk[i][:, 128 * m:128 * (m + 1)], cvw[i][:],
                          start=(i == 0), stop=(i == NDT - 1))
            vsb = kvr.tile([128, NKV * VW], BF16, tag="vsb", name="vsb",
                           bufs=2)
            for kh in range(NKV):
                ve.tensor_copy(vsb[:, VW * kh:VW * kh + HD],
                               pv[:, HD * kh:HD * (kh + 1)])
                ve.memset(vsb[:, VW * kh + HD:VW * (kh + 1)], 1.0)
            sy.dma_start(agv_in[128 * m:128 * (m + 1), :], vsb[:])

    gp.collective_compute("AllGather", ALU.bypass,
                          replica_groups=[list(range(NCORES))],
                          ins=[agk_in.opt()], outs=[agk_out.opt()])
    gp.collective_compute("AllGather", ALU.bypass,
                          replica_groups=[list(range(NCORES))],
                          ins=[agv_in.opt()], outs=[agv_out.opt()])

    # ===================== Stage B1: Q mix/proj/rope+norm ====================
    qa = es.enter_context(tc.tile_pool(name="qa", bufs=1))      # xmq: ->B4
    yap = es.enter_context(tc.tile_pool(name="yap", bufs=1))    # yall: ->B3
    xmq = [qa.tile([128, C], BF16, tag=f"xmq{i}", name=f"xmq{i}")
           for i in range(NDT)]
    yall = [yap.tile([128, C], BF16, tag=f"yall{i}", name=f"yall{i}")
            for i in range(NDT)]
    qrp = es.enter_context(tc.tile_pool(name="qrop", bufs=1))
    qro2 = [qrp.tile([64, 2 * C], BF16, tag=f"qro{m}", name=f"qro{m}")
            for m in range(NH // 2)]
    atk = es.enter_context(tc.tile_pool(name="atk", bufs=1))
    mask_sb = atk.tile([128, d_in["mask2"].shape[1]], BF16, tag="mask")
    kk, vv = [], []
    for r in range(NCORES):
        kt = atk.tile([64, NKV * KVBLK], BF16, tag=f"kk{r}", name=f"kk{r}")
        srck = agk_out[KVD * r:KVD * (r + 1), :].rearrange(
            "(a p) f -> p a f", a=NKV)
        sc.dma_start(kt[:].rearrange("p (a f) -> p a f", a=NKV), srck)
        kk.append(kt)
    with tc.tile_pool(name="qt", bufs=1) as qt, \
         tc.tile_pool(name="qrot", bufs=2) as qr2, \
         tc.tile_pool(name="qP", bufs=2, space="PSUM") as qp:
        for i in range(NDT):
            xq = qr2.tile([128, C], BF16, tag="xq", name="xq", bufs=2)
            sy.dma_start(xq[:], d_in["xqT"].ap()[128 * i:128 * (i + 1), :])
            x0q = qr2.tile([128, C], BF16, tag="x0q", name="x0q", bufs=2)
            sy.dma_start(x0q[:],
                         d_in["x0qT"].ap()[128 * i:128 * (i + 1), :])
            ve.tensor_scalar_mul(x0q[:], x0q[:], vecs["rm1v"][:, i:i + 1])
            ve.scalar_tensor_tensor(xmq[i][:], xq[:],
                                    vecs["rm0v"][:, i:i + 1],
                                    x0q[:], ALU.mult, ALU.add)

        for nm in ("cosq2", "sinq2"):
            t = qt.tile([128, C], BF16, tag=nm, name=nm)
            sy.dma_start(t[:], d_in[nm].ap())
            tbl[nm] = t

        def finish_q(m, psqs):
            qv = qro2[m][:].rearrange("p (f a) -> p f a", a=2)
            rope_norm(qr2, qp, psqs, C, tbl["cosq2"], tbl["sinq2"],
                      qg65[:, m:m + 1],
                      qv[:, :, 0], qv[:, :, 1], 0)

        pending = None
        for half in range(2):
            cqh = []
            for i in range(NDT):
                t = qt.tile([128, 512], BF16, tag=f"cqh{i}",
                            name=f"cqh{i}", bufs=2)
                sy.dma_start(t[:], d_in["cq_wT"].ap()
                             [128 * i:128 * (i + 1),
                              512 * half:512 * (half + 1)])
                cqh.append(t)
            for mm in range(4):
                m = 4 * half + mm
                psqs = []
                for (s, w) in _chunks(C):
                    psq = qp.tile([128, w], dt, tag="psq", name="psq",
                                  bufs=4)
                    for i in range(NDT):
                        pe.matmul(psq[:],
                                  cqh[i][:, 128 * mm:128 * (mm + 1)],
                                  xmq[i][:, s:s + w],
                                  start=(i == 0), stop=(i == NDT - 1))
                    psqs.append(psq)
                if pending is not None:
                    finish_q(*pending)
                pending = (m, psqs)
        finish_q(*pending)

    if d_dbg:
        for m in range(NH // 2):
            sy.dma_start(d_dbg["dbg_qro"].ap()[64 * m:64 * (m + 1), :],
                         qro2[m][:])
        for i in range(NDT):
            sy.dma_start(d_dbg["dbg_xmq"].ap()[128 * i:128 * (i + 1), :],
                         xmq[i][:])
        sy.dma_start(d_dbg["dbg_agk"].ap(), agk_out[:, :])
        sy.dma_start(d_dbg["dbg_agv"].ap(), agv_out[:, :])

    # ============ V bulk preload into SBUF (K preloaded at B1) ============
    for r in range(NCORES):
        vt = atk.tile([128, 4 * NKV * VW], BF16, tag=f"vv{r}", name=f"vv{r}")
        srcv = agv_out[KVBLK * r:KVBLK * (r + 1), :].rearrange(
            "(a p) f -> p a f", a=4)
        gp.dma_start(vt[:].rearrange("p (a f) -> p a f", a=4), srcv)
        vv.append(vt)
    gp.dma_start(mask_sb[:], d_in["mask2"].ap())

    # ========================= Stage B2: attention =========================
    # Per (b, kv-head, head-pair): both heads of the pair are computed in a
    # SINGLE score matmul per kv-tile entry (token-interleaved layout), then
    # exp (one call per strip group), then one mask-multiply that also
    # de-interleaves into per-head blocks for the V matmuls.  Groups are
    # software-pipelined so exp/mask of group g overlaps scores of g+1.
    with tc.tile_pool(name="at", bufs=1) as at, \
         tc.tile_pool(name="atP", bufs=2, space="PSUM") as atp:
        for b in range(B):
            last_j = groups[b][-1][0][-1][0]
            for kh in range(NKV):
                for hp in range(2):
                    m = 2 * kh + hp
                    h0 = 2 * m

                    def emit_v(gd, pys):
                        g, gwT, goff, sts, prb2 = gd
                        # exp de-interleaves: in (f a) token-interleaved,
                        # out (a f) per-head blocks
                        si = sts[:, 0:2 * gwT].rearrange(
                            "p (f a) -> p a f", a=2)
                        po = prb2[:, 0:2 * gwT].rearrange(
                            "p (a f) -> p a f", a=2)
                        sc.activation(po, si, ACT.Exp)
                        ve.tensor_mul(prb2[:, 0:2 * gwT], prb2[:, 0:2 * gwT],
                                      mask_sb[:, goff:goff + 2 * gwT])
                        for (j, Ws, Nw, ofs2) in g:
                            r = 4 * b + j // 4
                            vxj = vv[r][:, NKV * VW * (j % 4) + VW * kh:
                                        NKV * VW * (j % 4) + VW * (kh + 1)]
                            for hh in range(2):
                                pe.matmul(
                                    pys[hh][:, Ws:Ws + Nw], vxj,
                                    prb2[:, gwT * hh + ofs2 // 2:
                                         gwT * hh + ofs2 // 2 + Nw],
                                    start=False, stop=(j == last_j),
                                    skip_group_check=True)

                    pys = [atp.tile([VW, CB], dt, tag="py", name="py",
                                    bufs=2) for _ in range(2)]
                    for hh in range(2):
                        pe.matmul(pys[hh][:, 0:CB], zc65[:], zcb[:],
                                  start=True, stop=False,
                                  skip_group_check=True)
                    pend = None
                    for gi, (g, gwT) in enumerate(groups[b]):
                        goff = goffs[b][gi]
                        sts = atp.tile([128, 2 * gwT], dt, tag="st",
                                       name="st", bufs=3,
                                       padded_shape=[128, GRP2])
                        prb2 = at.tile([128, 2 * gwT], BF16, tag="prb2",
                                       name="prb2", bufs=3,
                                       padded_shape=[128, GRP2])
                        for (j, Ws, Nw, ofs2) in g:
                            r = 4 * b + j // 4
                            loc = KVBLK * kh + 128 * (j % 4)
                            ktj = kk[r][:, loc:loc + 128]
                            qs = qro2[m][:, 2 * (b * CB + Ws):
                                         2 * (b * CB + Ws + Nw)]
                            pe.matmul(sts[:, ofs2:ofs2 + 2 * Nw],
                                      ktj, qs, start=True, stop=True)
                        if pend is not None:
                            emit_v(pend, pys)
                        pend = (g, gwT, goff, sts, prb2)
                    emit_v(pend, pys)
                    for hh in range(2):
                        h = h0 + hh
                        rc0 = at.tile([1, CB], dt, tag="rc0", name="rc0",
                                      bufs=2)
                        ve.tensor_copy(rc0[:], pys[hh][64:65, :])
                        if d_dbg:
                            dce = at.tile([1, CB], BF16, tag="dce",
                                          name="dce", bufs=2)
                            ve.tensor_copy(dce[:], rc0[:])
                            sy.dma_start(
                                d_dbg["dbg_den"].ap()[h:h + 1,
                                                      b * CB:b * CB + CB],
                                dce[:])
                        rc = at.tile([1, CB], dt, tag="rc", name="rc",
                                     bufs=2)
                        ve.reciprocal_approx_fast(rc[:], rc0[:])
                        yb = at.tile([64, CB], dt, tag="yb", name="yb",
                                     bufs=2)
                        gp.partition_broadcast(yb[:], rc[0:1, :])
                        ve.tensor_mul(
                            yall[h // 2][64 * (h % 2):64 * (h % 2) + 64,
                                         b * CB:b * CB + CB],
                            pys[hh][0:64, :], yb[:])

    if d_dbg:
        for i in range(NDT):
            sy.dma_start(d_dbg["dbg_yall"].ap()[128 * i:128 * (i + 1), :],
                         yall[i][:])

    # ===================== Stage B3: out-proj + PID =====================
    with tc.tile_pool(name="pj", bufs=1) as pj, \
         tc.tile_pool(name="pjR", bufs=2) as pjr, \
         tc.tile_pool(name="pjP", bufs=2, space="PSUM") as pjp:
        for half in range(2):
            pjh = []
            for i in range(NDT):
                t = pj.tile([128, 512], BF16, tag=f"pjh{i}", name=f"pjh{i}",
                            bufs=2)
                sy.dma_start(t[:], d_in["proj_wT"].ap()
                             [128 * i:128 * (i + 1),
                              512 * half:512 * (half + 1)])
                pjh.append(t)
            for mm in range(4):
                m = 4 * half + mm
                velm = pjr.tile([128, C], BF16, tag="velm", name="velm",
                                bufs=2)
                sy.dma_start(velm[:],
                             d_in["velqT"].ap()[128 * m:128 * (m + 1), :])
                for (s, w) in _chunks(C):
                    pso = pjp.tile([128, w], dt, tag="pso", name="pso",
                                   bufs=2)
                    for i in range(NDT):
                        pe.matmul(pso[:], pjh[i][:, 128 * mm:128 * (mm + 1)],
                                  yall[i][:, s:s + w],
                                  start=(i == 0), stop=(i == NDT - 1))
                    ve.scalar_tensor_tensor(
                        xmq[m][:, s:s + w], pso[:],
                        vecs["ascalev"][:, m:m + 1],
                        xmq[m][:, s:s + w], ALU.mult, ALU.add)
                t2 = pjr.tile([128, C], dt, tag="t2", name="t2", bufs=2)
                sc.activation(t2[:], xmq[m][:], ACT.Identity,
                              bias=vecs["muc3v"][:, m:m + 1], scale=0.3)
                vn = pjr.tile([128, C], dt, tag="vn", name="vn", bufs=2)
                ve.scalar_tensor_tensor(vn[:], velm[:], 0.95, t2[:],
                                        ALU.mult, ALU.subtract)
                ve.tensor_scalar(vn[:], vn[:], 3.0, -3.0, ALU.min, ALU.max)
                sy.dma_start(d_vn.ap()[128 * m:128 * (m + 1), :], vn[:])
                ve.scalar_tensor_tensor(xmq[m][:], vn[:], 0.1 * 0.1,
                                        xmq[m][:], ALU.mult, ALU.add)

    # ============================ Stage B4: MoE ============================
    with tc.tile_pool(name="mo", bufs=1) as mo, \
         tc.tile_pool(name="moR", bufs=2) as mor, \
         tc.tile_pool(name="moP", bufs=2, space="PSUM") as mop:
        mn = rms_norm_T(mo, mor, mop, xmq, C, "mn")
        sg, hh_t = [], []
        for half in range(2):
            guh = []
            for i in range(NDT):
                t = mo.tile([128, 512], BF16, tag=f"guh{i}", name=f"guh{i}",
                            bufs=2)
                sy.dma_start(t[:], d_in["gu"].ap()
                             [128 * i:128 * (i + 1),
                              512 * half:512 * (half + 1)])
                guh.append(t)
            for mm in range(4):
                m = 4 * half + mm
                for (s, w) in _chunks(C):
                    psh = mop.tile([128, w], dt, tag="psh", name="psh",
                                   bufs=3)
                    for i in range(NDT):
                        pe.matmul(psh[:], guh[i][:, 128 * mm:128 * (mm + 1)],
                                  mn[i][:, s:s + w],
                                  start=(i == 0), stop=(i == NDT - 1))
                    if m < 4:
                        if s == 0:
                            sgm = mo.tile([128, C], dt, tag=f"sg{m}",
                                          name=f"sg{m}")
                            sg.append(sgm)
                        sc.activation(sg[m][:, s:s + w], psh[:], ACT.Silu)
                    else:
                        if s == 0:
                            hm = mo.tile([128, C], BF16, tag=f"hh{m - 4}",
                                         name=f"hh{m - 4}")
                            hh_t.append(hm)
                        ve.tensor_mul(hh_t[m - 4][:, s:s + w],
                                      sg[m - 4][:, s:s + w], psh[:])
        dnw = []
        for i2 in range(4):
            t = mo.tile([128, D], BF16, tag=f"dnw{i2}", name=f"dnw{i2}")
            sy.dma_start(t[:], d_in["dn"].ap()[128 * i2:128 * (i2 + 1), :])
            dnw.append(t)
        for m in range(NDT):
            xo = mor.tile([128, C], dt, tag="xo", name="xo", bufs=2)
            for (s, w) in _chunks(C):
                psm = mop.tile([128, w], dt, tag="psm", name="psm", bufs=2)
                for i2 in range(4):
                    pe.matmul(psm[:], dnw[i2][:, 128 * m:128 * (m + 1)],
                              hh_t[i2][:, s:s + w],
                              start=(i2 == 0), stop=(i2 == 3))
                ve.scalar_tensor_tensor(xo[:, s:s + w], psm[:],
                                        vecs["mscalev"][:, m:m + 1],
                                        xmq[m][:, s:s + w],
                                        ALU.mult, ALU.add)
            sy.dma_start(d_xout.ap()[128 * m:128 * (m + 1), :], xo[:])

    es.close()
